# revision 36
# baseline (speedup 1.0000x reference)
"""Trainium2 Bass kernel for nn_Attention_44830868635854.

Fused: 1x1-conv QKV -> depthwise 3x3 on q -> 8-head attention (softmax) ->
ReLU -> 1x1 proj -> GroupNorm(8).

Sharding: 8 cores = (batch b in 0..3) x (spatial half s in 0..1): each core
computes 1152 query pixels against the full 2304-key image for all 8 heads;
GroupNorm stats merge across the core pair via a tiny AllReduce.

The exp of 21.2M logits/core is the bottleneck (ACT alone ~140us), so key
double-chunks (i2) are split into classes:
  i2 0-4 (C2): ACT exp -> fp16, then (P-1) -> fp8e4 (Pool/DVE); AV runs as
      fp8 DoubleRow matmuls; the exact +sum(v) correction (host-computed)
      is applied at finalize. Storing P-1 keeps fp8 quantization noise ~10x
      below storing P (P clusters near 1: logits are small).
  i2 5 (C1): ACT exp -> fp16 P, fp16 AV.
  i2 6-7 (C4): Schraudolph exp on DVE: P's fp16 bit pattern is affine in
      the logit; one tensor_scalar with round-to-nearest uint16 convert.
  i2 8 (C5): dual Schraudolph (phase-offset pair, both fed to AV) --
      cancels most of the interpolation error.
Class scales are calibrated to exactly 1 so the softmax weighting stays
consistent. The denominator comes from a ones-column in V; normalization
is reciprocal + Sel-matmul replication. AV accumulates in PSUM bank 6.
"""

import numpy as np

import concourse.bass as bass
import concourse.mybir as mybir
import concourse.tile as tile
from concourse.tile import add_dep_helper
from concourse.bass_utils import run_bass_kernel_spmd

F32 = mybir.dt.float32
FP16 = mybir.dt.float16
U16 = mybir.dt.uint16
FP8 = mybir.dt.float8e4
AF = mybir.ActivationFunctionType
ALU = mybir.AluOpType
DR = mybir.MatmulPerfMode.DoubleRow

B, DIM, H, W = 4, 128, 48, 48
N = H * W            # 2304
ROWS_HALF = 24
NSL = ROWS_HALF * W  # 1152 per core
EPS = 1e-5
GN_DIV = 1.0 / (16.0 * N)

JT = [(0, 512), (512, 512), (1024, 128)]  # query tiles
JCOL = [0]
for _js, _nt in JT:
    JCOL.append(JCOL[-1] + (_nt + 255) // 256)
NI2 = 9                                   # key double-chunks of 256
# class per i2: 2=C2 fp8, 1=C1 act fp16, 4=C4 schraudolph, 5=C5 dual
CLS = [2, 2, 2, 1, 1, 1, 4, 4, 4]
# step-level interleave: 2 ACT-class steps then 1 DVE-class step
_A = [(i2, h) for i2 in (0, 4, 1, 2, 5, 3) for h in range(4)]
_D = [(i2, h) for i2 in (6, 8, 7) for h in range(4)]  # 8 is C4 now
STEPS = []
ia = idv = 0
while ia < len(_A) or idv < len(_D):
    for _ in range(2):
        if ia < len(_A):
            STEPS.append(_A[ia]); ia += 1
    if idv < len(_D):
        STEPS.append(_D[idv]); idv += 1
FIRST_I2H = {}
LAST_I2H = {}
for st in STEPS:
    pr = st[1] >> 1
    if pr not in FIRST_I2H:
        FIRST_I2H[pr] = st
    LAST_I2H[pr] = st
NC2KEYS = 256 * sum(1 for c in CLS if c == 2)

SCHR_A = 369.329926
SCHR_C1 = 15300.975
SCHR_CA = 14510.966
SCHR_CB = SCHR_CA - 512.0


def _split_multi_waits(nc):
    """walrus allows one sync-wait slot per lowered instruction; move extra
    waits onto standalone EventSemaphore instructions."""
    for func in nc.m.functions:
        for block in func.blocks:
            new_insts = []
            for inst in block.instructions:
                si = inst.sync_info
                waits = list(si.on_wait) if si is not None and si.on_wait else []
                if len(waits) > 1 and not isinstance(inst, mybir.InstEventSemaphore):
                    for k, w in enumerate(waits[:-1]):
                        new_insts.append(
                            mybir.InstEventSemaphore(
                                name=f"{inst.name}_wsplit{k}",
                                engine=inst.engine,
                                ins=[],
                                outs=[],
                                sync_info=mybir.SyncInfo(on_wait=[w], on_update=[]),
                            )
                        )
                    si.on_wait = waits[-1:]
                new_insts.append(inst)
            block.instructions[:] = new_insts


def _build(with_cc=True, debug=False):
    nc = bass.Bass()
    dt = nc.dram_tensor

    xb_d = dt("xb", [DIM, N], FP16, kind="ExternalInput")
    xq_d = dt("xq", [DIM, 26 * 50], FP16, kind="ExternalInput")
    wk_d = dt("wk", [2, DIM, 128], FP16, kind="ExternalInput")
    wq_d = dt("wq", [2, DIM, 128], FP16, kind="ExternalInput")
    wv_d = dt("wv", [DIM, 512], FP16, kind="ExternalInput")
    bvr_d = dt("bvr", [128, 512], F32, kind="ExternalInput")
    bvh_d = dt("bvh", [1, 512], FP16, kind="ExternalInput")
    sel_d = dt("sel", [DIM, 128], FP16, kind="ExternalInput")
    wpj_d = dt("wpj", [2, DIM, 128], FP16, kind="ExternalInput")
    bq_d = dt("bq", [128, 2], F32, kind="ExternalInput")
    bdw_d = dt("bdw", [128, 2], F32, kind="ExternalInput")
    dwt_d = dt("dwt", [128, 2, 9], F32, kind="ExternalInput")
    vsc_d = dt("vsc", [64, 4], F32, kind="ExternalInput")
    gab_d = dt("gab", [DIM, 2], F32, kind="ExternalInput")
    gsel_d = dt("gsel", [DIM, 8], F32, kind="ExternalInput")

    out_d = dt("out_half", [DIM, NSL], F32, kind="ExternalOutput")
    if debug:
        dbg_kg = dt("dbg_kg", [128, N], FP16, kind="ExternalOutput")
        dbg_qg = dt("dbg_qg", [128, NSL], FP16, kind="ExternalOutput")
        dbg_att = dt("dbg_att", [64, 2 * NSL], FP16, kind="ExternalOutput")
        dbg_o2 = dt("dbg_o2", [DIM, NSL], F32, kind="ExternalOutput")

    cc_in = dt("cc_in", [8, 2], F32)
    cc_out = dt("cc_out", [8, 2], F32)
    scratch_d = dt("scratch", [128, 1], F32)

    with tile.TileContext(nc) as tc:
        with (
            tc.tile_pool(name="persist", bufs=1) as pp,
            tc.tile_pool(name="fin", bufs=2) as fpool,
            tc.tile_pool(name="pt16p", bufs=9) as pt16p,
            tc.tile_pool(name="pt8p", bufs=9) as pt8p,
            tc.tile_pool(name="pbp", bufs=4) as pbp,
            tc.tile_pool(name="sq", bufs=2) as sqp,
            tc.tile_pool(name="lp", bufs=1, space="PSUM") as lpp,
        ):
            lpbig = lpp.tile([128, 8, 512], F32, tag="lpbig")
            psum_rr = [0]

            def psum_bank():
                b_ = psum_rr[0] % 8
                psum_rr[0] += 1
                return lpbig[:, b_ : b_ + 1, :]

            ctx_lp = nc.allow_low_precision(reason="fp16/fp8 attention")
            ctx_lp.__enter__()

            # ---- ACT exp table preload
            dummy = pp.tile([128, 1], F32, tag="dummy")
            nc.vector.memset(dummy, 0.0)
            nc.scalar.activation(out=dummy, in_=dummy, func=AF.Exp)
            nc.gpsimd.dma_start(out=scratch_d[:, :], in_=dummy)

            # ---- load inputs (fp16 operands come in pre-converted)
            xb = pp.tile([DIM, N], FP16, tag="xb")
            nc.sync.dma_start(out=xb, in_=xb_d[:, :])
            xq = pp.tile([DIM, 26 * 50], FP16, tag="xq")
            nc.sync.dma_start(out=xq, in_=xq_d[:, :])
            wk16, wq16 = [], []
            for g in range(2):
                t = pp.tile([DIM, 128], FP16, tag=f"wk{g}")
                nc.sync.dma_start(out=t, in_=wk_d[g, :, :])
                wk16.append(t)
                t = pp.tile([DIM, 128], FP16, tag=f"wq{g}")
                nc.sync.dma_start(out=t, in_=wq_d[g, :, :])
                wq16.append(t)

            wpjp16 = []
            for gp in range(4):
                t = pp.tile([64, 128], FP16, tag=f"wpjp{gp}")
                nc.sync.dma_start(out=t, in_=wpj_d[gp // 2, 64 * (gp % 2) : 64 * (gp % 2) + 64, :])
                wpjp16.append(t)
            wv16 = pp.tile([DIM, 512], FP16, tag="wv16")
            nc.sync.dma_start(out=wv16, in_=wv_d[:, :])
            bvr2 = pp.tile([128, 512], F32, tag="bvr2")
            nc.sync.dma_start(out=bvr2, in_=bvr_d[:, :])
            bvh = pp.tile([1, 512], FP16, tag="bvh")
            nc.sync.dma_start(out=bvh, in_=bvh_d[:, :])
            ones1 = pp.tile([1, 128], FP16, tag="ones1")
            nc.vector.memset(ones1, 1.0)
            selp16 = pp.tile([64, 64], FP16, tag="selp16")
            nc.sync.dma_start(out=selp16, in_=sel_d[0:64, 0:64])
            bq = pp.tile([128, 2], F32, tag="bq")
            nc.sync.dma_start(out=bq, in_=bq_d[:, :])
            bdw = pp.tile([128, 2], F32, tag="bdw")
            nc.sync.dma_start(out=bdw, in_=bdw_d[:, :])
            dwt = pp.tile([128, 2, 9], F32, tag="dwt")
            nc.sync.dma_start(out=dwt, in_=dwt_d[:, :, :])
            vsc = pp.tile([64, 4], F32, tag="vsc")
            nc.sync.dma_start(out=vsc, in_=vsc_d[:, :])
            gab = pp.tile([DIM, 2], F32, tag="gab")
            nc.sync.dma_start(out=gab, in_=gab_d[:, :])
            gsel = pp.tile([DIM, 8], F32, tag="gsel")
            nc.sync.dma_start(out=gsel, in_=gsel_d[:, :])

            # ---- K projection -> kg[g] fp16 [128, N]
            kg = []

            def emit_k(g):
                kt = pp.tile([128, N], FP16, tag=f"kg{g}", name=f"kg{g}")
                for j0 in range(0, N, 512):
                    n = min(512, N - j0)
                    ps = psum_bank()
                    nc.tensor.matmul(
                        out=ps[:, 0, 0:n], lhsT=wk16[g], rhs=xb[:, j0 : j0 + n],
                        start=True, stop=True, skip_group_check=True,
                    )
                    nc.scalar.copy(out=kt[:, j0 : j0 + n], in_=ps[:, 0, 0:n])
                kg.append(kt)


            # ---- Q: qpre = Wq xq + bq (f32), then 3x3 dw conv on DVE
            qg = []

            def emit_q(g):
                qpre = pp.tile([128, 26, 50], F32, tag=f"qpre{g}")
                qpre_f = qpre.rearrange("p a b -> p (a b)")
                for c0, cn in ((0, 500), (500, 500), (1000, 300)):
                    ps = psum_bank()
                    nc.tensor.matmul(
                        out=ps[:, 0, 0:cn], lhsT=wq16[g],
                        rhs=xq[:, c0 : c0 + cn], start=True, stop=True,
                        skip_group_check=True,
                    )
                    nc.scalar.activation(
                        out=qpre_f[:, c0 : c0 + cn],
                        in_=ps[:, 0, 0:cn], func=AF.Identity,
                        bias=bq[:, g : g + 1], scale=1.0,
                    )
                qa = pp.tile([128, 24, 48], F32, tag=f"qa{g}")
                qb = pp.tile([128, 24, 48], F32, tag=f"qb{g}")
                qt = pp.tile([128, NSL], FP16, tag=f"qg{g}", name=f"qg{g}")
                cur, nxt = qa, qb
                eng = nc.vector
                for tap in range(9):
                    ty, tx = tap // 3, tap % 3
                    win = qpre[:, ty : ty + 24, tx : tx + 48]
                    w_ap = dwt[:, g, tap : tap + 1]
                    if tap == 0:
                        eng.tensor_scalar(
                            out=cur, in0=win, scalar1=w_ap,
                            scalar2=bdw[:, g : g + 1],
                            op0=ALU.mult, op1=ALU.add,
                        )
                    elif tap < 8:
                        eng.scalar_tensor_tensor(
                            out=nxt, in0=win, scalar=w_ap, in1=cur,
                            op0=ALU.mult, op1=ALU.add,
                        )
                        cur, nxt = nxt, cur
                    else:
                        eng.scalar_tensor_tensor(
                            out=qt.rearrange("p (a b) -> p a b", b=48),
                            in0=win, scalar=w_ap, in1=cur,
                            op0=ALU.mult, op1=ALU.add,
                        )
                qg.append(qt)

            emit_k(0)
            emit_q(0)


            # V-proj psum on banks 2..7 only (attention set 0 = banks
            # 0,1 starts immediately); chunks in step-consumption order.
            V_ORDER = [0, 4, 1, 6, 2, 8, 5, 3, 7]
            psum_rr_v = [0]

            def psum_bank_v():
                b_ = 2 + psum_rr_v[0] % 6
                psum_rr_v[0] += 1
                return lpbig[:, b_ : b_ + 1, :]

            # ---- V projection -> vt8 (fp8) / vt16 (fp16); wv is
            # pre-padded to the 64-wide head-slot layout (zeros in the
            # pair-mate half) so the drain is one plain TT add per chunk.
            vt8 = [None] * NI2
            vt16 = [None] * (2 * NI2)
            for i2 in range(NI2):
                c = CLS[i2]
                if c == 2:
                    vt8[i2] = pp.tile([128, 2, 512], FP8, tag=f"vt8_{i2}",
                                      name=f"vt8_{i2}")
                for t in range(2):
                    i = 2 * i2 + t
                    ps = psum_bank_v()
                    nc.tensor.matmul(
                        out=ps[:, 0, 0:512],
                        lhsT=xb[:, i * 128 : (i + 1) * 128],
                        rhs=wv16, start=True, stop=False, skip_group_check=True,
                    )
                    nc.tensor.matmul(
                        out=ps[:, 0, 0:512], lhsT=ones1, rhs=bvh,
                        start=False, stop=True, skip_group_check=True,
                    )
                    if c == 2:
                        dst = vt8[i2][:, t, :]
                    else:
                        vt = pp.tile([128, 512], FP16, tag=f"vt16_{i}",
                                     name=f"vt16_{i}")
                        vt16[i] = vt
                        dst = vt
                    if i % 4 == 0:
                        nc.scalar.copy(out=dst, in_=ps[:, 0, 0:512])
                    else:
                        nc.vector.tensor_copy(out=dst, in_=ps[:, 0, 0:512])

            emit_k(1)
            emit_q(1)

            # ---- attention state (pair-split layout, partitions 0..63)
            att = []
            for g in range(2):
                a = pp.tile([64, 2, NSL], FP16, tag=f"att{g}")
                nc.vector.memset(a, 0.0)
                att.append(a)
            scopy = pp.tile([64, 2, 512], FP16, tag="scopy")
            o2 = pp.tile([DIM, NSL], F32, tag="o2")
            s1p = pp.tile([DIM, 5], F32, tag="s1p")
            s2p = pp.tile([DIM, 5], F32, tag="s2p")

            op2_rr = [0]

            def op2_engine():
                op2_rr[0] += 1
                return nc.gpsimd if (op2_rr[0] % 2) != 0 else nc.vector

            accp = [lpbig[0:64, 6, :], lpbig[0:64, 7, :]]  # per pair

            step_ctr = [0]
            av_queue = []

            def emit_av(item):
                g, h, i2, NT, pt = item
                pa_i, hl = h >> 1, h & 1
                slot = 4 * g + h
                acc = accp[pa_i]
                cls = CLS[i2]
                first = (i2, h) == FIRST_I2H[pa_i]
                last = (i2, h) == LAST_I2H[pa_i]
                if cls == 2:
                    nq = (NT + 255) // 256
                    for q in range(nq):
                        qn = min(256, NT - 256 * q)
                        nc.tensor.matmul(
                            out=acc[:, 256 * q : 256 * q + qn],
                            lhsT=vt8[i2][:, :, 64 * slot : 64 * slot + 64],
                            rhs=pt[:, :, 256 * q : 256 * q + qn],
                            start=(first and q == 0),
                            stop=(last and q == nq - 1),
                            perf_mode=DR, skip_group_check=True,
                            tile_position=(0, 0),
                        )
                elif cls in (1, 4):
                    for t in range(2):
                        nc.tensor.matmul(
                            out=acc[:, 0:NT],
                            lhsT=vt16[2 * i2 + t][:, 64 * slot : 64 * slot + 64],
                            rhs=pt[:, t, 0:NT],
                            start=(first and t == 0),
                            stop=(last and t == 1),
                            skip_group_check=True,
                            tile_position=(0, 0),
                        )
                else:  # C5 dual
                    pa, pb = pt
                    for k, p_ in enumerate((pa, pb)):
                        for t in range(2):
                            nc.tensor.matmul(
                                out=acc[:, 0:NT],
                                lhsT=vt16[2 * i2 + t][:, 64 * slot : 64 * slot + 64],
                                rhs=p_[:, t, 0:NT],
                                start=(first and k == 0 and t == 0),
                                stop=(last and k == 1 and t == 1),
                                skip_group_check=True,
                                tile_position=(0, 0),
                            )

            for jh, (js, NT) in enumerate(JT):
                jsl = slice(js, js + NT)
                for g in range(2):
                    for si_, (i2, h) in enumerate(STEPS):
                        cls = CLS[i2]
                        if True:
                            s = step_ctr[0] % 3
                            step_ctr[0] += 1
                            for t in range(2):
                                i = 2 * i2 + t
                                nc.tensor.matmul(
                                    out=lpbig[:, 2 * s + t, 0:NT],
                                    lhsT=kg[g][32 * h : 32 * h + 16,
                                               i * 128 : (i + 1) * 128],
                                    rhs=qg[g][32 * h : 32 * h + 16, jsl],
                                    start=True, stop=True,
                                    tile_position=(32 * h, 0),
                                    skip_group_check=True,
                                )
                            lin = lpbig[:, 2 * s : 2 * s + 2, 0:NT]
                            if cls == 2:
                                pt16 = pt16p.tile([128, 2, 512], FP16,
                                                  tag="pt16")
                                nc.scalar.activation(
                                    out=pt16[:, :, 0:NT], in_=lin,
                                    func=AF.Exp, scale=0.25,
                                )
                                pt8 = pt8p.tile([128, 2, 512], FP8, tag="pt8")
                                op2_engine().tensor_scalar(
                                    out=pt8[:, :, 0:NT], in0=pt16[:, :, 0:NT],
                                    scalar1=1.0, scalar2=-1.0,
                                    op0=ALU.mult, op1=ALU.add,
                                )
                                pt = pt8
                            elif cls == 1:
                                pt16 = pt16p.tile([128, 2, 512], FP16,
                                                  tag="pt16")
                                nc.scalar.activation(
                                    out=pt16[:, :, 0:NT], in_=lin,
                                    func=AF.Exp, scale=0.25,
                                )
                                pt = pt16
                            elif cls == 4:
                                ptu = pt16p.tile([128, 2, 512], U16,
                                                 tag="pt16")
                                nc.vector.tensor_scalar(
                                    out=ptu[:, :, 0:NT], in0=lin,
                                    scalar1=SCHR_A, scalar2=SCHR_C1,
                                    op0=ALU.mult, op1=ALU.add,
                                )
                                pt = ptu.bitcast(FP16)
                            else:  # C5
                                pa = pt16p.tile([128, 2, 512], U16,
                                                tag="pt16")
                                nc.vector.tensor_scalar(
                                    out=pa[:, :, 0:NT], in0=lin,
                                    scalar1=SCHR_A, scalar2=SCHR_CA,
                                    op0=ALU.mult, op1=ALU.add,
                                )
                                pb = pbp.tile([128, 2, 512], U16, tag="pb")
                                nc.vector.tensor_scalar(
                                    out=pb[:, :, 0:NT], in0=lin,
                                    scalar1=SCHR_A, scalar2=SCHR_CB,
                                    op0=ALU.mult, op1=ALU.add,
                                )
                                pt = (pa.bitcast(FP16), pb.bitcast(FP16))
                            av_queue.append((g, h, i2, NT, pt))
                            if len(av_queue) > 10:
                                emit_av(av_queue.pop(0))
                    while av_queue:
                        emit_av(av_queue.pop(0))
                    # ---- finalize (g, j): pair-split
                    fin1 = fpool.tile([64, 2, 512], F32, tag="fin1")
                    for pa_i in range(2):
                        nc.vector.tensor_scalar(
                            out=fin1[:, pa_i, 0:NT], in0=accp[pa_i][:, 0:NT],
                            scalar1=vsc[:, 2 * g + pa_i : 2 * g + pa_i + 1],
                            scalar2=0.01, op0=ALU.add, op1=ALU.max,
                        )
                    nc.vector.reciprocal(
                        out=scopy[:, :, 0:NT], in_=fin1[:, :, 0:NT]
                    )
                    # replicate 1/S (rows 0,32 of each pair) via Sel matmuls
                    for pa_i in range(2):
                        nc.tensor.matmul(
                            out=lpbig[0:64, 6 + pa_i, 0:NT], lhsT=selp16,
                            rhs=scopy[:, pa_i, 0:NT], start=True, stop=True,
                            skip_group_check=True, tile_position=(0, 0),
                        )
                    nc.vector.tensor_mul(
                        out=att[g][:, :, js : js + NT],
                        in0=fin1[:, :, 0:NT], in1=lpbig[0:64, 6:8, 0:NT],
                    )
                    if g == 1:
                        nq = (NT + 255) // 256
                        for q in range(nq):
                            qn = min(256, NT - 256 * q)
                            col = JCOL[jh] + q
                            qs = slice(js + 256 * q, js + 256 * q + qn)
                            for gg in range(2):
                                for pa_i in range(2):
                                    nc.tensor.matmul(
                                        out=lpbig[:, 6, 0:qn],
                                        lhsT=wpjp16[2 * gg + pa_i],
                                        rhs=att[gg][:, pa_i, qs],
                                        start=(gg == 0 and pa_i == 0),
                                        stop=(gg == 1 and pa_i == 1),
                                        skip_group_check=True,
                                    )
                            o2s = o2[:, qs]
                            nc.scalar.activation(
                                out=o2s, in_=lpbig[:, 6, 0:qn],
                                func=AF.Copy, accum_out=s1p[:, col : col + 1],
                            )
                            sq = sqp.tile([128, 256], F32, tag="sq")
                            nc.scalar.activation(
                                out=sq[:, 0:qn], in_=o2s, func=AF.Square,
                                accum_out=s2p[:, col : col + 1],
                            )

            # ---- GroupNorm
            s12 = pp.tile([DIM, 2], F32, tag="s12")
            nc.vector.tensor_reduce(
                out=s12[:, 0:1], in_=s1p, op=ALU.add, axis=mybir.AxisListType.X
            )
            nc.vector.tensor_reduce(
                out=s12[:, 1:2], in_=s2p, op=ALU.add, axis=mybir.AxisListType.X
            )
            s12r = pp.tile([DIM, 2], mybir.dt.float32r, tag="s12r")
            nc.vector.tensor_copy(out=s12r, in_=s12)
            gselr = pp.tile([DIM, 8], mybir.dt.float32r, tag="gselr")
            nc.vector.tensor_copy(out=gselr, in_=gsel)
            gp = psum_bank()
            nc.tensor.matmul(
                out=gp[0:8, 0, 0:2], lhsT=gselr, rhs=s12r, start=True,
                stop=True, skip_group_check=True,
            )
            gst = pp.tile([8, 2], F32, tag="gst")
            nc.vector.tensor_copy(out=gst, in_=gp[0:8, 0, 0:2])
            ccw = nc.gpsimd.dma_start(out=cc_in[:, :], in_=gst)
            if with_cc:
                cci = nc.gpsimd.collective_compute(
                    "AllReduce", ALU.add,
                    ins=[cc_in[:, :]], outs=[cc_out[:, :]],
                    replica_groups=[[0, 1], [2, 3], [4, 5], [6, 7]],
                )
            else:
                cci = nc.gpsimd.dma_start(out=cc_out[:, :], in_=cc_in[:, :])
            add_dep_helper(cci.ins, ccw.ins, reason="cc_in RAW")
            gch = pp.tile([DIM, 2], F32, tag="gch")
            ccr = nc.gpsimd.dma_start(
                out=gch,
                in_=bass.AP(
                    tensor=cc_out[:, :].tensor, offset=0,
                    ap=[[2, 8], [0, 16], [1, 2]],
                ),
            )
            add_dep_helper(ccr.ins, cci.ins, reason="cc_out RAW")
            mu = pp.tile([DIM, 1], F32, tag="mu")
            nc.vector.tensor_scalar_mul(out=mu, in0=gch[:, 0:1], scalar1=GN_DIV)
            ex2 = pp.tile([DIM, 1], F32, tag="ex2")
            nc.vector.tensor_scalar_mul(out=ex2, in0=gch[:, 1:2], scalar1=GN_DIV)
            mu2 = pp.tile([DIM, 1], F32, tag="mu2")
            nc.vector.tensor_mul(out=mu2, in0=mu, in1=mu)
            var = pp.tile([DIM, 1], F32, tag="var")
            nc.vector.tensor_sub(out=var, in0=ex2, in1=mu2)
            epst = pp.tile([DIM, 1], F32, tag="epst")
            nc.vector.memset(epst, EPS)
            lnv = pp.tile([DIM, 1], F32, tag="lnv")
            nc.scalar.activation(out=lnv, in_=var, func=AF.Ln, bias=epst)
            rstd = pp.tile([DIM, 1], F32, tag="rstd")
            nc.scalar.activation(out=rstd, in_=lnv, func=AF.Exp, scale=-0.5)
            A_ = pp.tile([DIM, 1], F32, tag="A")
            nc.vector.tensor_mul(out=A_, in0=rstd, in1=gab[:, 0:1])
            muA = pp.tile([DIM, 1], F32, tag="muA")
            nc.vector.tensor_mul(out=muA, in0=mu, in1=A_)
            Bc = pp.tile([DIM, 1], F32, tag="Bc")
            nc.vector.tensor_sub(out=Bc, in0=gab[:, 1:2], in1=muA)
            of = pp.tile([DIM, NSL], F32, tag="of")
            nc.gpsimd.tensor_scalar(
                out=of, in0=o2, scalar1=A_, scalar2=Bc,
                op0=ALU.mult, op1=ALU.add,
            )
            nc.sync.dma_start(out=out_d[:, :], in_=of)
            if debug:
                nc.sync.dma_start(out=dbg_kg[:, :], in_=kg[0])
                nc.sync.dma_start(out=dbg_qg[:, :], in_=qg[0])
                nc.sync.dma_start(
                    out=dbg_att[:, :],
                    in_=att[0].rearrange("p a b -> p (a b)"))
                nc.sync.dma_start(out=dbg_o2[:, :], in_=o2)
            ctx_lp.__exit__(None, None, None)

    _split_multi_waits(nc)
    return nc


_CACHE = {}


def _prep(w_qkv, b_qkv, w_dw, b_dw, w_proj, gn_w, gn_b):
    """Host-side weight layout prep (group g, slot jj in 0..3, dim d)."""
    ch = lambda g, jj, d: (4 * g + jj) * 16 + d
    wk = np.zeros((2, DIM, 128), np.float16)
    wq = np.zeros((2, DIM, 128), np.float16)
    wv = np.zeros((DIM, 512), np.float16)
    bvr = np.zeros((128, 512), np.float32)
    bq = np.zeros((128, 2), np.float32)
    bdw = np.zeros((128, 2), np.float32)
    dwt = np.zeros((128, 2, 9), np.float32)
    wpj = np.zeros((2, DIM, 128), np.float16)
    for g in range(2):
        for jj in range(4):
            for d in range(16):
                c = ch(g, jj, d)
                p = 32 * jj + d
                wk[g, :, p] = w_qkv[128 + c, :]
                wq[g, :, p] = w_qkv[c, :]
                s_no = 4 * g + jj
                col = 64 * s_no + 32 * (jj % 2) + d + 1
                wv[:, col] = w_qkv[256 + c, :]
                bvr[:, col] = b_qkv[256 + c]
                bq[p, g] = b_qkv[c]
                bdw[p, g] = b_dw[c]
                for tap in range(9):
                    dwt[p, g, tap] = w_dw[c, 0, tap // 3, tap % 3]
                wpj[g, p + 1, :] = w_proj[:, c]
            bvr[:, 64 * (4 * g + jj) + 32 * (jj % 2)] = 1.0
    sel = np.zeros((DIM, 128), np.float16)
    for o in range(128):
        sel[32 * (o // 32), o] = 1.0
    gab = np.stack([gn_w, gn_b], axis=1).astype(np.float32)
    gsel = np.zeros((DIM, 8), np.float32)
    for c in range(DIM):
        gsel[c, c // 16] = 1.0
    vpad = -np.linalg.solve(w_qkv[0:128, :].astype(np.float64),
                            b_qkv[0:128].astype(np.float64)).astype(np.float32)
    wdict = dict(wk=wk, wq=wq, wv=wv, bvr=bvr, bq=bq, bdw=bdw, dwt=dwt,
                 wpj=wpj, sel=sel, gab=gab, gsel=gsel)
    return wdict, vpad


def kernel(x, w_qkv, b_qkv, w_dw, b_dw, w_proj, gn_w, gn_b):
    x = np.asarray(x, np.float32)
    w_qkv = np.asarray(w_qkv, np.float32)
    b_qkv = np.asarray(b_qkv, np.float32)
    w_dw = np.asarray(w_dw, np.float32)
    b_dw = np.asarray(b_dw, np.float32)
    w_proj = np.asarray(w_proj, np.float32)
    gn_w = np.asarray(gn_w, np.float32)
    gn_b = np.asarray(gn_b, np.float32)

    weights, vpad = _prep(w_qkv, b_qkv, w_dw, b_dw, w_proj, gn_w, gn_b)

    if "nc" not in _CACHE:
        _CACHE["nc"] = _build()
    nc = _CACHE["nc"]

    ch = lambda g, jj, d: (4 * g + jj) * 16 + d
    wv_q = w_qkv[256:384, :]
    bv = b_qkv[256:384]

    in_maps = []
    for c in range(8):
        b, s = c // 2, c % 2
        xb = x[b].reshape(DIM, N)
        xq = np.empty((DIM, 26, 50), np.float32)
        xq[:, :, :] = vpad[:, None, None]
        xv = x[b]
        if s == 0:
            xq[:, 1:26, 1:49] = xv[:, 0:25, :]
        else:
            xq[:, 0:25, 1:49] = xv[:, 23:48, :]
        xb16 = xb.astype(np.float16)
        # vsum correction for C2 chunks, from the fp16 x actually used
        xsum = xb16.astype(np.float32)[:, 0:NC2KEYS].sum(axis=1)
        vsum_ch = wv_q @ xsum + NC2KEYS * bv
        vsc = np.ones((64, 4), np.float32)
        for g in range(2):
            for jj in range(4):
                pa_i, hl = jj >> 1, jj & 1
                vsc[32 * hl, 2 * g + pa_i] = float(NC2KEYS)
                for d in range(16):
                    vsc[32 * hl + 1 + d, 2 * g + pa_i] = vsum_ch[ch(g, jj, d)]
        m = {"xb": np.ascontiguousarray(xb16),
             "xq": xq.reshape(DIM, 26 * 50).astype(np.float16),
             "vsc": vsc,
             "bvh": weights["bvr"][0:1, :].astype(np.float16)}
        m.update(weights)
        in_maps.append(m)

    res = run_bass_kernel_spmd(nc, in_maps, core_ids=list(range(8)))

    out = np.empty((B, DIM, H, W), np.float32)
    for c in range(8):
        b, s = c // 2, c % 2
        out[b, :, 24 * s : 24 * s + 24, :] = res.results[c]["out_half"].reshape(
            DIM, ROWS_HALF, W
        )
    return out


# revision 37
# speedup vs baseline: 1.0191x; 1.0191x over previous
"""Trainium2 Bass kernel for nn_Attention_44830868635854.

Fused: 1x1-conv QKV -> depthwise 3x3 on q -> 8-head attention (softmax) ->
ReLU -> 1x1 proj -> GroupNorm(8).

Sharding: 8 cores = (batch b in 0..3) x (spatial half s in 0..1): each core
computes 1152 query pixels against the full 2304-key image for all 8 heads;
GroupNorm stats merge across the core pair via a tiny AllReduce.

The exp of 21.2M logits/core is the bottleneck (ACT alone ~140us), so key
double-chunks (i2) are split into classes:
  i2 0-4 (C2): ACT exp -> fp16, then (P-1) -> fp8e4 (Pool/DVE); AV runs as
      fp8 DoubleRow matmuls; the exact +sum(v) correction (host-computed)
      is applied at finalize. Storing P-1 keeps fp8 quantization noise ~10x
      below storing P (P clusters near 1: logits are small).
  i2 5 (C1): ACT exp -> fp16 P, fp16 AV.
  i2 6-7 (C4): Schraudolph exp on DVE: P's fp16 bit pattern is affine in
      the logit; one tensor_scalar with round-to-nearest uint16 convert.
  i2 8 (C5): dual Schraudolph (phase-offset pair, both fed to AV) --
      cancels most of the interpolation error.
Class scales are calibrated to exactly 1 so the softmax weighting stays
consistent. The denominator comes from a ones-column in V; normalization
is reciprocal + Sel-matmul replication. AV accumulates in PSUM bank 6.
"""

import numpy as np

import concourse.bass as bass
import concourse.mybir as mybir
import concourse.tile as tile
from concourse.tile import add_dep_helper
from concourse.bass_utils import run_bass_kernel_spmd

F32 = mybir.dt.float32
FP16 = mybir.dt.float16
U16 = mybir.dt.uint16
FP8 = mybir.dt.float8e4
AF = mybir.ActivationFunctionType
ALU = mybir.AluOpType
DR = mybir.MatmulPerfMode.DoubleRow

B, DIM, H, W = 4, 128, 48, 48
N = H * W            # 2304
ROWS_HALF = 24
NSL = ROWS_HALF * W  # 1152 per core
EPS = 1e-5
GN_DIV = 1.0 / (16.0 * N)

JT = [(0, 512), (512, 512), (1024, 128)]  # query tiles
JCOL = [0]
for _js, _nt in JT:
    JCOL.append(JCOL[-1] + (_nt + 255) // 256)
NI2 = 9                                   # key double-chunks of 256
# class per i2: 2=C2 fp8, 1=C1 act fp16, 4=C4 schraudolph, 5=C5 dual
CLS = [2, 2, 2, 2, 1, 1, 4, 4, 4]
# step-level interleave: 2 ACT-class steps then 1 DVE-class step
_A = [(i2, h) for i2 in (0, 4, 1, 2, 5, 3) for h in range(4)]
_D = [(i2, h) for i2 in (6, 8, 7) for h in range(4)]  # 8 is C4 now
STEPS = []
ia = idv = 0
while ia < len(_A) or idv < len(_D):
    for _ in range(2):
        if ia < len(_A):
            STEPS.append(_A[ia]); ia += 1
    if idv < len(_D):
        STEPS.append(_D[idv]); idv += 1
FIRST_I2H = {}
LAST_I2H = {}
for st in STEPS:
    pr = st[1] >> 1
    if pr not in FIRST_I2H:
        FIRST_I2H[pr] = st
    LAST_I2H[pr] = st
NC2KEYS = 256 * sum(1 for c in CLS if c == 2)

SCHR_A = 369.329926
SCHR_C1 = 15300.975
SCHR_CA = 14510.966
SCHR_CB = SCHR_CA - 512.0


def _split_multi_waits(nc):
    """walrus allows one sync-wait slot per lowered instruction; move extra
    waits onto standalone EventSemaphore instructions."""
    for func in nc.m.functions:
        for block in func.blocks:
            new_insts = []
            for inst in block.instructions:
                si = inst.sync_info
                waits = list(si.on_wait) if si is not None and si.on_wait else []
                if len(waits) > 1 and not isinstance(inst, mybir.InstEventSemaphore):
                    for k, w in enumerate(waits[:-1]):
                        new_insts.append(
                            mybir.InstEventSemaphore(
                                name=f"{inst.name}_wsplit{k}",
                                engine=inst.engine,
                                ins=[],
                                outs=[],
                                sync_info=mybir.SyncInfo(on_wait=[w], on_update=[]),
                            )
                        )
                    si.on_wait = waits[-1:]
                new_insts.append(inst)
            block.instructions[:] = new_insts


def _build(with_cc=True, debug=False):
    nc = bass.Bass()
    dt = nc.dram_tensor

    xb_d = dt("xb", [DIM, N], FP16, kind="ExternalInput")
    xq_d = dt("xq", [DIM, 26 * 50], FP16, kind="ExternalInput")
    wk_d = dt("wk", [2, DIM, 128], FP16, kind="ExternalInput")
    wq_d = dt("wq", [2, DIM, 128], FP16, kind="ExternalInput")
    wv_d = dt("wv", [DIM, 512], FP16, kind="ExternalInput")
    bvr_d = dt("bvr", [128, 512], F32, kind="ExternalInput")
    bvh_d = dt("bvh", [1, 512], FP16, kind="ExternalInput")
    sel_d = dt("sel", [DIM, 128], FP16, kind="ExternalInput")
    wpj_d = dt("wpj", [2, DIM, 128], FP16, kind="ExternalInput")
    bq_d = dt("bq", [128, 2], F32, kind="ExternalInput")
    bdw_d = dt("bdw", [128, 2], F32, kind="ExternalInput")
    dwt_d = dt("dwt", [128, 2, 9], F32, kind="ExternalInput")
    vsc_d = dt("vsc", [64, 4], F32, kind="ExternalInput")
    gab_d = dt("gab", [DIM, 2], F32, kind="ExternalInput")
    gsel_d = dt("gsel", [DIM, 8], F32, kind="ExternalInput")

    out_d = dt("out_half", [DIM, NSL], F32, kind="ExternalOutput")
    if debug:
        dbg_kg = dt("dbg_kg", [128, N], FP16, kind="ExternalOutput")
        dbg_qg = dt("dbg_qg", [128, NSL], FP16, kind="ExternalOutput")
        dbg_att = dt("dbg_att", [64, 2 * NSL], FP16, kind="ExternalOutput")
        dbg_o2 = dt("dbg_o2", [DIM, NSL], F32, kind="ExternalOutput")

    cc_in = dt("cc_in", [8, 2], F32)
    cc_out = dt("cc_out", [8, 2], F32)
    scratch_d = dt("scratch", [128, 1], F32)

    with tile.TileContext(nc) as tc:
        with (
            tc.tile_pool(name="persist", bufs=1) as pp,
            tc.tile_pool(name="fin", bufs=2) as fpool,
            tc.tile_pool(name="pt16p", bufs=9) as pt16p,
            tc.tile_pool(name="pt8p", bufs=9) as pt8p,
            tc.tile_pool(name="pbp", bufs=4) as pbp,
            tc.tile_pool(name="sq", bufs=2) as sqp,
            tc.tile_pool(name="lp", bufs=1, space="PSUM") as lpp,
        ):
            lpbig = lpp.tile([128, 8, 512], F32, tag="lpbig")
            psum_rr = [0]

            def psum_bank():
                b_ = psum_rr[0] % 8
                psum_rr[0] += 1
                return lpbig[:, b_ : b_ + 1, :]

            ctx_lp = nc.allow_low_precision(reason="fp16/fp8 attention")
            ctx_lp.__enter__()

            # ---- ACT exp table preload
            dummy = pp.tile([128, 1], F32, tag="dummy")
            nc.vector.memset(dummy, 0.0)
            nc.scalar.activation(out=dummy, in_=dummy, func=AF.Exp)
            nc.gpsimd.dma_start(out=scratch_d[:, :], in_=dummy)

            # ---- load inputs (fp16 operands come in pre-converted)
            xb = pp.tile([DIM, N], FP16, tag="xb")
            nc.sync.dma_start(out=xb, in_=xb_d[:, :])
            xq = pp.tile([DIM, 26 * 50], FP16, tag="xq")
            nc.sync.dma_start(out=xq, in_=xq_d[:, :])
            wk16, wq16 = [], []
            for g in range(2):
                t = pp.tile([DIM, 128], FP16, tag=f"wk{g}")
                nc.sync.dma_start(out=t, in_=wk_d[g, :, :])
                wk16.append(t)
                t = pp.tile([DIM, 128], FP16, tag=f"wq{g}")
                nc.sync.dma_start(out=t, in_=wq_d[g, :, :])
                wq16.append(t)

            wpjp16 = []
            for gp in range(4):
                t = pp.tile([64, 128], FP16, tag=f"wpjp{gp}")
                nc.sync.dma_start(out=t, in_=wpj_d[gp // 2, 64 * (gp % 2) : 64 * (gp % 2) + 64, :])
                wpjp16.append(t)
            wv16 = pp.tile([DIM, 512], FP16, tag="wv16")
            nc.sync.dma_start(out=wv16, in_=wv_d[:, :])
            bvr2 = pp.tile([128, 512], F32, tag="bvr2")
            nc.sync.dma_start(out=bvr2, in_=bvr_d[:, :])
            bvh = pp.tile([1, 512], FP16, tag="bvh")
            nc.sync.dma_start(out=bvh, in_=bvh_d[:, :])
            ones1 = pp.tile([1, 128], FP16, tag="ones1")
            nc.vector.memset(ones1, 1.0)
            selp16 = pp.tile([64, 64], FP16, tag="selp16")
            nc.sync.dma_start(out=selp16, in_=sel_d[0:64, 0:64])
            bq = pp.tile([128, 2], F32, tag="bq")
            nc.sync.dma_start(out=bq, in_=bq_d[:, :])
            bdw = pp.tile([128, 2], F32, tag="bdw")
            nc.sync.dma_start(out=bdw, in_=bdw_d[:, :])
            dwt = pp.tile([128, 2, 9], F32, tag="dwt")
            nc.sync.dma_start(out=dwt, in_=dwt_d[:, :, :])
            vsc = pp.tile([64, 4], F32, tag="vsc")
            nc.sync.dma_start(out=vsc, in_=vsc_d[:, :])
            gab = pp.tile([DIM, 2], F32, tag="gab")
            nc.sync.dma_start(out=gab, in_=gab_d[:, :])
            gsel = pp.tile([DIM, 8], F32, tag="gsel")
            nc.sync.dma_start(out=gsel, in_=gsel_d[:, :])

            # ---- K projection -> kg[g] fp16 [128, N]
            kg = []

            def emit_k(g):
                kt = pp.tile([128, N], FP16, tag=f"kg{g}", name=f"kg{g}")
                for j0 in range(0, N, 512):
                    n = min(512, N - j0)
                    ps = psum_bank()
                    nc.tensor.matmul(
                        out=ps[:, 0, 0:n], lhsT=wk16[g], rhs=xb[:, j0 : j0 + n],
                        start=True, stop=True, skip_group_check=True,
                    )
                    nc.scalar.copy(out=kt[:, j0 : j0 + n], in_=ps[:, 0, 0:n])
                kg.append(kt)


            # ---- Q: qpre = Wq xq + bq (f32), then 3x3 dw conv on DVE
            qg = []

            def emit_q(g):
                qpre = pp.tile([128, 26, 50], F32, tag=f"qpre{g}")
                qpre_f = qpre.rearrange("p a b -> p (a b)")
                for c0, cn in ((0, 500), (500, 500), (1000, 300)):
                    ps = psum_bank()
                    nc.tensor.matmul(
                        out=ps[:, 0, 0:cn], lhsT=wq16[g],
                        rhs=xq[:, c0 : c0 + cn], start=True, stop=True,
                        skip_group_check=True,
                    )
                    nc.scalar.activation(
                        out=qpre_f[:, c0 : c0 + cn],
                        in_=ps[:, 0, 0:cn], func=AF.Identity,
                        bias=bq[:, g : g + 1], scale=1.0,
                    )
                qa = pp.tile([128, 24, 48], F32, tag=f"qa{g}")
                qb = pp.tile([128, 24, 48], F32, tag=f"qb{g}")
                qt = pp.tile([128, NSL], FP16, tag=f"qg{g}", name=f"qg{g}")
                cur, nxt = qa, qb
                eng = nc.vector
                for tap in range(9):
                    ty, tx = tap // 3, tap % 3
                    win = qpre[:, ty : ty + 24, tx : tx + 48]
                    w_ap = dwt[:, g, tap : tap + 1]
                    if tap == 0:
                        eng.tensor_scalar(
                            out=cur, in0=win, scalar1=w_ap,
                            scalar2=bdw[:, g : g + 1],
                            op0=ALU.mult, op1=ALU.add,
                        )
                    elif tap < 8:
                        eng.scalar_tensor_tensor(
                            out=nxt, in0=win, scalar=w_ap, in1=cur,
                            op0=ALU.mult, op1=ALU.add,
                        )
                        cur, nxt = nxt, cur
                    else:
                        eng.scalar_tensor_tensor(
                            out=qt.rearrange("p (a b) -> p a b", b=48),
                            in0=win, scalar=w_ap, in1=cur,
                            op0=ALU.mult, op1=ALU.add,
                        )
                qg.append(qt)

            emit_k(0)
            emit_q(0)


            # V-proj psum on banks 2..7 only (attention set 0 = banks
            # 0,1 starts immediately); chunks in step-consumption order.
            V_ORDER = [0, 4, 1, 6, 2, 8, 5, 3, 7]
            psum_rr_v = [0]

            def psum_bank_v():
                b_ = 2 + psum_rr_v[0] % 6
                psum_rr_v[0] += 1
                return lpbig[:, b_ : b_ + 1, :]

            # ---- V projection -> vt8 (fp8) / vt16 (fp16); wv is
            # pre-padded to the 64-wide head-slot layout (zeros in the
            # pair-mate half) so the drain is one plain TT add per chunk.
            vt8 = [None] * NI2
            vt16 = [None] * (2 * NI2)
            for i2 in range(NI2):
                c = CLS[i2]
                if c == 2:
                    vt8[i2] = pp.tile([128, 2, 512], FP8, tag=f"vt8_{i2}",
                                      name=f"vt8_{i2}")
                for t in range(2):
                    i = 2 * i2 + t
                    ps = psum_bank_v()
                    nc.tensor.matmul(
                        out=ps[:, 0, 0:512],
                        lhsT=xb[:, i * 128 : (i + 1) * 128],
                        rhs=wv16, start=True, stop=False, skip_group_check=True,
                    )
                    nc.tensor.matmul(
                        out=ps[:, 0, 0:512], lhsT=ones1, rhs=bvh,
                        start=False, stop=True, skip_group_check=True,
                    )
                    if c == 2:
                        dst = vt8[i2][:, t, :]
                    else:
                        vt = pp.tile([128, 512], FP16, tag=f"vt16_{i}",
                                     name=f"vt16_{i}")
                        vt16[i] = vt
                        dst = vt
                    if i % 4 == 0:
                        nc.scalar.copy(out=dst, in_=ps[:, 0, 0:512])
                    else:
                        nc.vector.tensor_copy(out=dst, in_=ps[:, 0, 0:512])

            emit_k(1)
            emit_q(1)

            # ---- attention state (pair-split layout, partitions 0..63)
            att = []
            for g in range(2):
                a = pp.tile([64, 2, NSL], FP16, tag=f"att{g}")
                nc.vector.memset(a, 0.0)
                att.append(a)
            scopy = pp.tile([64, 2, 512], FP16, tag="scopy")
            o2 = pp.tile([DIM, NSL], F32, tag="o2")
            s1p = pp.tile([DIM, 5], F32, tag="s1p")
            s2p = pp.tile([DIM, 5], F32, tag="s2p")

            op2_rr = [0]

            def op2_engine():
                op2_rr[0] += 1
                return nc.gpsimd if (op2_rr[0] % 2) != 0 else nc.vector

            accp = [lpbig[0:64, 6, :], lpbig[0:64, 7, :]]  # per pair

            step_ctr = [0]
            av_queue = []

            def emit_av(item):
                g, h, i2, NT, pt = item
                pa_i, hl = h >> 1, h & 1
                slot = 4 * g + h
                acc = accp[pa_i]
                cls = CLS[i2]
                first = (i2, h) == FIRST_I2H[pa_i]
                last = (i2, h) == LAST_I2H[pa_i]
                if cls == 2:
                    nq = (NT + 255) // 256
                    for q in range(nq):
                        qn = min(256, NT - 256 * q)
                        nc.tensor.matmul(
                            out=acc[:, 256 * q : 256 * q + qn],
                            lhsT=vt8[i2][:, :, 64 * slot : 64 * slot + 64],
                            rhs=pt[:, :, 256 * q : 256 * q + qn],
                            start=(first and q == 0),
                            stop=(last and q == nq - 1),
                            perf_mode=DR, skip_group_check=True,
                            tile_position=(0, 0),
                        )
                elif cls in (1, 4):
                    for t in range(2):
                        nc.tensor.matmul(
                            out=acc[:, 0:NT],
                            lhsT=vt16[2 * i2 + t][:, 64 * slot : 64 * slot + 64],
                            rhs=pt[:, t, 0:NT],
                            start=(first and t == 0),
                            stop=(last and t == 1),
                            skip_group_check=True,
                            tile_position=(0, 0),
                        )
                else:  # C5 dual
                    pa, pb = pt
                    for k, p_ in enumerate((pa, pb)):
                        for t in range(2):
                            nc.tensor.matmul(
                                out=acc[:, 0:NT],
                                lhsT=vt16[2 * i2 + t][:, 64 * slot : 64 * slot + 64],
                                rhs=p_[:, t, 0:NT],
                                start=(first and k == 0 and t == 0),
                                stop=(last and k == 1 and t == 1),
                                skip_group_check=True,
                                tile_position=(0, 0),
                            )

            for jh, (js, NT) in enumerate(JT):
                jsl = slice(js, js + NT)
                for g in range(2):
                    for si_, (i2, h) in enumerate(STEPS):
                        cls = CLS[i2]
                        if True:
                            s = step_ctr[0] % 3
                            step_ctr[0] += 1
                            for t in range(2):
                                i = 2 * i2 + t
                                nc.tensor.matmul(
                                    out=lpbig[:, 2 * s + t, 0:NT],
                                    lhsT=kg[g][32 * h : 32 * h + 16,
                                               i * 128 : (i + 1) * 128],
                                    rhs=qg[g][32 * h : 32 * h + 16, jsl],
                                    start=True, stop=True,
                                    tile_position=(32 * h, 0),
                                    skip_group_check=True,
                                )
                            lin = lpbig[:, 2 * s : 2 * s + 2, 0:NT]
                            if cls == 2:
                                pt16 = pt16p.tile([128, 2, 512], FP16,
                                                  tag="pt16")
                                nc.scalar.activation(
                                    out=pt16[:, :, 0:NT], in_=lin,
                                    func=AF.Exp, scale=0.25,
                                )
                                pt8 = pt8p.tile([128, 2, 512], FP8, tag="pt8")
                                op2_engine().tensor_scalar(
                                    out=pt8[:, :, 0:NT], in0=pt16[:, :, 0:NT],
                                    scalar1=1.0, scalar2=-1.0,
                                    op0=ALU.mult, op1=ALU.add,
                                )
                                pt = pt8
                            elif cls == 1:
                                pt16 = pt16p.tile([128, 2, 512], FP16,
                                                  tag="pt16")
                                nc.scalar.activation(
                                    out=pt16[:, :, 0:NT], in_=lin,
                                    func=AF.Exp, scale=0.25,
                                )
                                pt = pt16
                            elif cls == 4:
                                ptu = pt16p.tile([128, 2, 512], U16,
                                                 tag="pt16")
                                nc.vector.tensor_scalar(
                                    out=ptu[:, :, 0:NT], in0=lin,
                                    scalar1=SCHR_A, scalar2=SCHR_C1,
                                    op0=ALU.mult, op1=ALU.add,
                                )
                                pt = ptu.bitcast(FP16)
                            else:  # C5
                                pa = pt16p.tile([128, 2, 512], U16,
                                                tag="pt16")
                                nc.vector.tensor_scalar(
                                    out=pa[:, :, 0:NT], in0=lin,
                                    scalar1=SCHR_A, scalar2=SCHR_CA,
                                    op0=ALU.mult, op1=ALU.add,
                                )
                                pb = pbp.tile([128, 2, 512], U16, tag="pb")
                                nc.vector.tensor_scalar(
                                    out=pb[:, :, 0:NT], in0=lin,
                                    scalar1=SCHR_A, scalar2=SCHR_CB,
                                    op0=ALU.mult, op1=ALU.add,
                                )
                                pt = (pa.bitcast(FP16), pb.bitcast(FP16))
                            av_queue.append((g, h, i2, NT, pt))
                            if len(av_queue) > 10:
                                emit_av(av_queue.pop(0))
                    while av_queue:
                        emit_av(av_queue.pop(0))
                    # ---- finalize (g, j): pair-split
                    fin1 = fpool.tile([64, 2, 512], F32, tag="fin1")
                    for pa_i in range(2):
                        nc.vector.tensor_scalar(
                            out=fin1[:, pa_i, 0:NT], in0=accp[pa_i][:, 0:NT],
                            scalar1=vsc[:, 2 * g + pa_i : 2 * g + pa_i + 1],
                            scalar2=0.01, op0=ALU.add, op1=ALU.max,
                        )
                    nc.vector.reciprocal(
                        out=scopy[:, :, 0:NT], in_=fin1[:, :, 0:NT]
                    )
                    # replicate 1/S (rows 0,32 of each pair) via Sel matmuls
                    for pa_i in range(2):
                        nc.tensor.matmul(
                            out=lpbig[0:64, 6 + pa_i, 0:NT], lhsT=selp16,
                            rhs=scopy[:, pa_i, 0:NT], start=True, stop=True,
                            skip_group_check=True, tile_position=(0, 0),
                        )
                    nc.vector.tensor_mul(
                        out=att[g][:, :, js : js + NT],
                        in0=fin1[:, :, 0:NT], in1=lpbig[0:64, 6:8, 0:NT],
                    )
                    if g == 1:
                        nq = (NT + 255) // 256
                        for q in range(nq):
                            qn = min(256, NT - 256 * q)
                            col = JCOL[jh] + q
                            qs = slice(js + 256 * q, js + 256 * q + qn)
                            for gg in range(2):
                                for pa_i in range(2):
                                    nc.tensor.matmul(
                                        out=lpbig[:, 6, 0:qn],
                                        lhsT=wpjp16[2 * gg + pa_i],
                                        rhs=att[gg][:, pa_i, qs],
                                        start=(gg == 0 and pa_i == 0),
                                        stop=(gg == 1 and pa_i == 1),
                                        skip_group_check=True,
                                    )
                            o2s = o2[:, qs]
                            nc.scalar.activation(
                                out=o2s, in_=lpbig[:, 6, 0:qn],
                                func=AF.Copy, accum_out=s1p[:, col : col + 1],
                            )
                            sq = sqp.tile([128, 256], F32, tag="sq")
                            nc.scalar.activation(
                                out=sq[:, 0:qn], in_=o2s, func=AF.Square,
                                accum_out=s2p[:, col : col + 1],
                            )

            # ---- GroupNorm
            s12 = pp.tile([DIM, 2], F32, tag="s12")
            nc.vector.tensor_reduce(
                out=s12[:, 0:1], in_=s1p, op=ALU.add, axis=mybir.AxisListType.X
            )
            nc.vector.tensor_reduce(
                out=s12[:, 1:2], in_=s2p, op=ALU.add, axis=mybir.AxisListType.X
            )
            s12r = pp.tile([DIM, 2], mybir.dt.float32r, tag="s12r")
            nc.vector.tensor_copy(out=s12r, in_=s12)
            gselr = pp.tile([DIM, 8], mybir.dt.float32r, tag="gselr")
            nc.vector.tensor_copy(out=gselr, in_=gsel)
            gp = psum_bank()
            nc.tensor.matmul(
                out=gp[0:8, 0, 0:2], lhsT=gselr, rhs=s12r, start=True,
                stop=True, skip_group_check=True,
            )
            gst = pp.tile([8, 2], F32, tag="gst")
            nc.vector.tensor_copy(out=gst, in_=gp[0:8, 0, 0:2])
            ccw = nc.gpsimd.dma_start(out=cc_in[:, :], in_=gst)
            if with_cc:
                cci = nc.gpsimd.collective_compute(
                    "AllReduce", ALU.add,
                    ins=[cc_in[:, :]], outs=[cc_out[:, :]],
                    replica_groups=[[0, 1], [2, 3], [4, 5], [6, 7]],
                )
            else:
                cci = nc.gpsimd.dma_start(out=cc_out[:, :], in_=cc_in[:, :])
            add_dep_helper(cci.ins, ccw.ins, reason="cc_in RAW")
            gch = pp.tile([DIM, 2], F32, tag="gch")
            ccr = nc.gpsimd.dma_start(
                out=gch,
                in_=bass.AP(
                    tensor=cc_out[:, :].tensor, offset=0,
                    ap=[[2, 8], [0, 16], [1, 2]],
                ),
            )
            add_dep_helper(ccr.ins, cci.ins, reason="cc_out RAW")
            mu = pp.tile([DIM, 1], F32, tag="mu")
            nc.vector.tensor_scalar_mul(out=mu, in0=gch[:, 0:1], scalar1=GN_DIV)
            ex2 = pp.tile([DIM, 1], F32, tag="ex2")
            nc.vector.tensor_scalar_mul(out=ex2, in0=gch[:, 1:2], scalar1=GN_DIV)
            mu2 = pp.tile([DIM, 1], F32, tag="mu2")
            nc.vector.tensor_mul(out=mu2, in0=mu, in1=mu)
            var = pp.tile([DIM, 1], F32, tag="var")
            nc.vector.tensor_sub(out=var, in0=ex2, in1=mu2)
            epst = pp.tile([DIM, 1], F32, tag="epst")
            nc.vector.memset(epst, EPS)
            lnv = pp.tile([DIM, 1], F32, tag="lnv")
            nc.scalar.activation(out=lnv, in_=var, func=AF.Ln, bias=epst)
            rstd = pp.tile([DIM, 1], F32, tag="rstd")
            nc.scalar.activation(out=rstd, in_=lnv, func=AF.Exp, scale=-0.5)
            A_ = pp.tile([DIM, 1], F32, tag="A")
            nc.vector.tensor_mul(out=A_, in0=rstd, in1=gab[:, 0:1])
            muA = pp.tile([DIM, 1], F32, tag="muA")
            nc.vector.tensor_mul(out=muA, in0=mu, in1=A_)
            Bc = pp.tile([DIM, 1], F32, tag="Bc")
            nc.vector.tensor_sub(out=Bc, in0=gab[:, 1:2], in1=muA)
            of = pp.tile([DIM, NSL], F32, tag="of")
            for h0 in (0, 384, 768):
                nc.gpsimd.tensor_scalar(
                    out=of[:, h0 : h0 + 384], in0=o2[:, h0 : h0 + 384],
                    scalar1=A_, scalar2=Bc, op0=ALU.mult, op1=ALU.add,
                )
                nc.sync.dma_start(
                    out=out_d[:, h0 : h0 + 384], in_=of[:, h0 : h0 + 384]
                )
            if debug:
                nc.sync.dma_start(out=dbg_kg[:, :], in_=kg[0])
                nc.sync.dma_start(out=dbg_qg[:, :], in_=qg[0])
                nc.sync.dma_start(
                    out=dbg_att[:, :],
                    in_=att[0].rearrange("p a b -> p (a b)"))
                nc.sync.dma_start(out=dbg_o2[:, :], in_=o2)
            ctx_lp.__exit__(None, None, None)

    _split_multi_waits(nc)
    return nc


_CACHE = {}


def _prep(w_qkv, b_qkv, w_dw, b_dw, w_proj, gn_w, gn_b):
    """Host-side weight layout prep (group g, slot jj in 0..3, dim d)."""
    ch = lambda g, jj, d: (4 * g + jj) * 16 + d
    wk = np.zeros((2, DIM, 128), np.float16)
    wq = np.zeros((2, DIM, 128), np.float16)
    wv = np.zeros((DIM, 512), np.float16)
    bvr = np.zeros((128, 512), np.float32)
    bq = np.zeros((128, 2), np.float32)
    bdw = np.zeros((128, 2), np.float32)
    dwt = np.zeros((128, 2, 9), np.float32)
    wpj = np.zeros((2, DIM, 128), np.float16)
    for g in range(2):
        for jj in range(4):
            for d in range(16):
                c = ch(g, jj, d)
                p = 32 * jj + d
                wk[g, :, p] = w_qkv[128 + c, :]
                wq[g, :, p] = w_qkv[c, :]
                s_no = 4 * g + jj
                col = 64 * s_no + 32 * (jj % 2) + d + 1
                wv[:, col] = w_qkv[256 + c, :]
                bvr[:, col] = b_qkv[256 + c]
                bq[p, g] = b_qkv[c]
                bdw[p, g] = b_dw[c]
                for tap in range(9):
                    dwt[p, g, tap] = w_dw[c, 0, tap // 3, tap % 3]
                wpj[g, p + 1, :] = w_proj[:, c]
            bvr[:, 64 * (4 * g + jj) + 32 * (jj % 2)] = 1.0
    sel = np.zeros((DIM, 128), np.float16)
    for o in range(128):
        sel[32 * (o // 32), o] = 1.0
    gab = np.stack([gn_w, gn_b], axis=1).astype(np.float32)
    gsel = np.zeros((DIM, 8), np.float32)
    for c in range(DIM):
        gsel[c, c // 16] = 1.0
    vpad = -np.linalg.solve(w_qkv[0:128, :].astype(np.float64),
                            b_qkv[0:128].astype(np.float64)).astype(np.float32)
    wdict = dict(wk=wk, wq=wq, wv=wv, bvr=bvr, bq=bq, bdw=bdw, dwt=dwt,
                 wpj=wpj, sel=sel, gab=gab, gsel=gsel)
    return wdict, vpad


def kernel(x, w_qkv, b_qkv, w_dw, b_dw, w_proj, gn_w, gn_b):
    x = np.asarray(x, np.float32)
    w_qkv = np.asarray(w_qkv, np.float32)
    b_qkv = np.asarray(b_qkv, np.float32)
    w_dw = np.asarray(w_dw, np.float32)
    b_dw = np.asarray(b_dw, np.float32)
    w_proj = np.asarray(w_proj, np.float32)
    gn_w = np.asarray(gn_w, np.float32)
    gn_b = np.asarray(gn_b, np.float32)

    weights, vpad = _prep(w_qkv, b_qkv, w_dw, b_dw, w_proj, gn_w, gn_b)

    if "nc" not in _CACHE:
        _CACHE["nc"] = _build()
    nc = _CACHE["nc"]

    ch = lambda g, jj, d: (4 * g + jj) * 16 + d
    wv_q = w_qkv[256:384, :]
    bv = b_qkv[256:384]

    in_maps = []
    for c in range(8):
        b, s = c // 2, c % 2
        xb = x[b].reshape(DIM, N)
        xq = np.empty((DIM, 26, 50), np.float32)
        xq[:, :, :] = vpad[:, None, None]
        xv = x[b]
        if s == 0:
            xq[:, 1:26, 1:49] = xv[:, 0:25, :]
        else:
            xq[:, 0:25, 1:49] = xv[:, 23:48, :]
        xb16 = xb.astype(np.float16)
        # vsum correction for C2 chunks, from the fp16 x actually used
        xsum = xb16.astype(np.float32)[:, 0:NC2KEYS].sum(axis=1)
        vsum_ch = wv_q @ xsum + NC2KEYS * bv
        vsc = np.ones((64, 4), np.float32)
        for g in range(2):
            for jj in range(4):
                pa_i, hl = jj >> 1, jj & 1
                vsc[32 * hl, 2 * g + pa_i] = float(NC2KEYS)
                for d in range(16):
                    vsc[32 * hl + 1 + d, 2 * g + pa_i] = vsum_ch[ch(g, jj, d)]
        m = {"xb": np.ascontiguousarray(xb16),
             "xq": xq.reshape(DIM, 26 * 50).astype(np.float16),
             "vsc": vsc,
             "bvh": weights["bvr"][0:1, :].astype(np.float16)}
        m.update(weights)
        in_maps.append(m)

    res = run_bass_kernel_spmd(nc, in_maps, core_ids=list(range(8)))

    out = np.empty((B, DIM, H, W), np.float32)
    for c in range(8):
        b, s = c // 2, c % 2
        out[b, :, 24 * s : 24 * s + 24, :] = res.results[c]["out_half"].reshape(
            DIM, ROWS_HALF, W
        )
    return out
